# revision 32
# baseline (speedup 1.0000x reference)
"""CenterCTCLoss Trainium2 kernel.

Strategy (data-parallel over batch, 8 cores, 512 rows each):
  The reference computation collapses to three segment statistics per core:
    counts[c] = sum of mask where labels==c
    S[c,d]    = sum of mask*features where labels==c
    SQ[c,d]   = sum of mask*features^2 where labels==c   (q = SQ.sum())
  where mask is the CTC char mask derived from argmax(preds).  Everything
  else (loss, centers update) is O(n_class*feat) math done on host from the
  8 per-core partial sums:
    upd[c]      = ALPHA*(counts[c]*centers[c] - S[c]) / (1+counts[c])
    new_centers = centers - upd
    loss        = 0.5*(sum_c counts[c]*|centers[c]|^2 - 2*<centers,S> + q)

  On device per chunk of 128 batch rows (batch on partitions), software-
  pipelined two stages deep so DMA/GPSIMD of chunk c+1 overlap the
  mask/scatter/matmul tail of chunk c:
    m[b,t]  = reduce_max over classes                     (DVE)
    dd      = bf16(preds - m) <= 0, == 0 only at argmax   (GPSIMD, exact)
    zmax[t] = max_c(dd_t + dd_{t+1})                      (DVE; == 0 iff the
              argmax repeats: a sum of two non-positives is 0 iff both are 0,
              which is exact in any float format)
    mask    = (dd[...,BLANK] != 0) * (zmax != 0)          (tiny)
    moh[p, (t%16)*85+lab] = mask[p,t]                     (GPSIMD local_scatter
              builds the masked one-hot; indices depend only on labels)
    psumA[85,64] += moh_t.T @ x_t   (bf16 PE, fp32 accum; one-hot and mask are
    psumB[85,65] += moh_t.T @ [x_t^2 | 1]    exact in bf16, features round)
"""

import sys

sys.path.insert(0, "/opt/trn_rl_repo")

from contextlib import ExitStack

import numpy as np

import concourse.bacc as bacc
import concourse.bass as bass
import concourse.mybir as mybir
import concourse.tile as tile
from concourse.bass_utils import run_bass_kernel_spmd

N_CLASS = 85
BLANK = 84
ALPHA = 0.05
B, T, D = 4096, 96, 64
NCORES = 8
BS = B // NCORES  # 512 batch rows per core
TH = T // 2  # half-chunk along t for DMA tiles

f32 = mybir.dt.float32
i32 = mybir.dt.int32
i16 = mybir.dt.int16
bf16 = mybir.dt.bfloat16

FDP = T * N_CLASS  # 8160 preds elems per row
NOUT = 2 * D + 1  # 129 output cols: S | SQ | counts


def build_program(bs: int = BS):
    nchunk = bs // 128
    nc = bacc.Bacc(
        "TRN2", target_bir_lowering=False, debug=False, num_devices=NCORES
    )
    preds = nc.dram_tensor("preds", [bs, T, N_CLASS], f32, kind="ExternalInput").ap()
    feats = nc.dram_tensor("features", [bs, T, D], f32, kind="ExternalInput").ap()
    labels = nc.dram_tensor("labels", [bs, T], i32, kind="ExternalInput").ap()
    out = nc.dram_tensor("out", [N_CLASS, NOUT], f32, kind="ExternalOutput").ap()

    CP = N_CLASS + 1  # 86: class dim padded so bf16 shifted views stay 4B-aligned

    with tile.TileContext(nc, pool_alloc_mode="queue") as tc, ExitStack() as ctx:
        consts = ctx.enter_context(tc.tile_pool(name="consts", bufs=1))
        ppool = ctx.enter_context(tc.tile_pool(name="ppool", bufs=8))
        xpool = ctx.enter_context(tc.tile_pool(name="xpool", bufs=2))
        work = ctx.enter_context(tc.tile_pool(name="work", bufs=1))
        work2 = ctx.enter_context(tc.tile_pool(name="work2", bufs=2))
        mmin = ctx.enter_context(tc.tile_pool(name="mmin", bufs=2))
        small = ctx.enter_context(tc.tile_pool(name="small", bufs=4))
        psum_pool = ctx.enter_context(
            tc.tile_pool(name="psum", bufs=1, space="PSUM")
        )

        # per-t scatter constant: tb85[p,t] = (t%16)*85
        TB = 16  # t-block size for local_scatter (16*85=1360 elems < 2047)
        NB = T // TB
        tb85i = consts.tile([128, T], i16)
        nc.gpsimd.iota(
            tb85i[:], pattern=[[0, NB], [N_CLASS, TB]], base=0, channel_multiplier=0
        )
        tb85f = consts.tile([128, T], f32)
        nc.scalar.copy(tb85f[:], tb85i[:])

        psumA = psum_pool.tile([N_CLASS, D], f32)  # S
        psumB = psum_pool.tile([N_CLASS, D + 1], f32)  # SQ | counts

        def repdet_half(st, half):
            # repeated argmax for one t-half of the previous chunk:
            # zmax[t] = max_c(dd_t + dd_{t+1}) == 0 iff argmax repeats (a sum
            # of two non-positives is 0 iff both are 0 -- exact in bf16).
            # The shifted add runs in place over dd: in1 reads stay 86
            # elements ahead of the writes, so old values are consumed first.
            ddf = st["dd"][:].rearrange("p t c -> p (t c)")
            nhalf = (T - 1) // 2  # 47; halves are 47 and 48 wide
            w0, w1 = ((0, nhalf), (nhalf, T - 1))[half]
            nc.vector.tensor_add(
                ddf[:, w0 * CP : w1 * CP],
                ddf[:, w0 * CP : w1 * CP],
                ddf[:, (w0 + 1) * CP : (w1 + 1) * CP],
            )
            nc.vector.tensor_reduce(
                st["zmax"][:, w0:w1],
                ddf[:, w0 * CP : w1 * CP].rearrange("p (t c) -> p t c", c=CP),
                axis=mybir.AxisListType.X,
                op=mybir.AluOpType.max,
            )

        def stage1a(c, prev):
            brange = slice(c * 128, (c + 1) * 128)
            # preds path: load; dd[p,t,c] = preds - max <= 0, 0 only at argmax
            dd = work2.tile([128, T, CP], bf16, tag="dd")
            m = small.tile([128, T], f32, tag="m")
            nc.gpsimd.memset(dd[:, :, N_CLASS], -1e30)
            TQ = T // 4
            for h in range(4):
                ts_ = slice(h * TQ, (h + 1) * TQ)
                ph = ppool.tile([128, TQ, N_CLASS], f32, tag="ph")
                nc.sync.dma_start(ph[:], preds[brange, ts_, :])

                nc.vector.tensor_reduce(
                    m[:, ts_], ph[:], axis=mybir.AxisListType.X, op=mybir.AluOpType.max
                )
                mb = m[:, ts_].unsqueeze(2).broadcast_to([128, TQ, N_CLASS])
                nc.gpsimd.tensor_tensor(
                    dd[:, ts_, 0:N_CLASS], ph[:], mb, op=mybir.AluOpType.subtract
                )
                # interleave the previous chunk's repeat-detection halves so
                # the DVE has ready work while this chunk's DMAs land
                if prev is not None and h in (1, 3):
                    repdet_half(prev, h // 2)
            # blank-argmax indicator only needs dd
            inv84 = small.tile([128, T], f32, tag="inv84")
            nc.vector.tensor_single_scalar(
                inv84[:], dd[:, :, BLANK], 0.0, op=mybir.AluOpType.not_equal
            )
            zmax = small.tile([128, T], f32, tag="zmax")
            nc.vector.memset(zmax[:, T - 1 : T], -1.0)
            return dict(dd=dd, inv84=inv84, zmax=zmax)

        def stage1b(c, st):
            brange = slice(c * 128, (c + 1) * 128)
            # features/labels path, only needed by the stage2 matmuls
            xb = mmin.tile([128, T, D], bf16, tag="xb")
            xsq = mmin.tile([128, T, D + 1], bf16, tag="xsq")
            labI = small.tile([128, T], i32, tag="labI")
            nc.sync.dma_start(labI[:], labels[brange, :])
            for h in range(2):
                ts_ = slice(h * TH, (h + 1) * TH)
                xh = xpool.tile([128, TH, D], f32, tag="xh")
                nc.sync.dma_start(xh[:], feats[brange, ts_, :])
                # features: bf16 copy and squares (ScalarE)
                nc.scalar.copy(xb[:, ts_, :], xh[:])
                nc.scalar.square(xsq[:, ts_, 0:D], xh[:])
            nc.vector.memset(xsq[:, :, D], 1.0)
            labf = small.tile([128, T], f32, tag="labf")
            nc.scalar.copy(labf[:], labI[:])
            idx16 = small.tile([128, T], i16, tag="idx16")
            nc.vector.tensor_add(idx16[:], tb85f[:], labf[:])
            st.update(xb=xb, xsq=xsq, idx16=idx16)
            return st

        def stage2(c, st):
            xb, xsq = st["xb"], st["xsq"]
            inv84, idx16, zmax = st["inv84"], st["idx16"], st["zmax"]

            # mask = (dd[..,BLANK] != 0) * (zmax != 0), scattered as DATA:
            # moh[p, (t%16)*85+lab] = mask[p,t] (indices never masked, always
            # distinct within a 16-t block)
            maskb = small.tile([128, T], bf16, tag="maskb")
            nc.vector.scalar_tensor_tensor(
                maskb[:], zmax[:], 0.0, inv84[:],
                op0=mybir.AluOpType.not_equal, op1=mybir.AluOpType.mult,
            )

            # masked one-hot (bf16) via gpsimd local scatter
            moh = work.tile([128, FDP], bf16, tag="moh")
            moh3 = moh[:].rearrange("p (t c) -> p t c", c=N_CLASS)
            for blk in range(NB):
                nc.gpsimd.local_scatter(
                    moh[:, blk * TB * N_CLASS : (blk + 1) * TB * N_CLASS],
                    maskb[:, blk * TB : (blk + 1) * TB],
                    idx16[:, blk * TB : (blk + 1) * TB],
                    channels=128,
                    num_elems=TB * N_CLASS,
                    num_idxs=TB,
                )

            # segment sums via PE
            for t in range(T):
                first = c == 0 and t == 0
                last = c == nchunk - 1 and t == T - 1
                nc.tensor.matmul(
                    psumA[:], moh3[:, t, :], xb[:, t, :], start=first, stop=last
                )
                nc.tensor.matmul(
                    psumB[:], moh3[:, t, :], xsq[:, t, :], start=first, stop=last
                )

        # software pipeline: the preds path of chunk c+1 (DMA, max-reduce,
        # gpsimd subtract) is emitted before stage2 of chunk c so it queues
        # ahead of the heavy DVE tail work; the features path follows.
        pending = None
        for c in range(nchunk + 1):
            if c < nchunk:
                st = stage1a(c, pending)
            else:
                for half in (0, 1):
                    repdet_half(pending, half)
            if pending is not None:
                stage2(c - 1, pending)
            if c < nchunk:
                stage1b(c, st)
            pending = st if c < nchunk else None

        outT = consts.tile([N_CLASS, NOUT], f32)
        nc.scalar.copy(outT[:, 0:D], psumA[:])
        nc.scalar.copy(outT[:, D : 2 * D + 1], psumB[:])
        nc.sync.dma_start(out[:], outT[:])

    nc.compile()
    return nc


_prog_cache: dict[int, object] = {}


def _get_program(bs: int = BS):
    if bs not in _prog_cache:
        _prog_cache[bs] = build_program(bs)
    return _prog_cache[bs]


def combine_partials(parts: np.ndarray, centers: np.ndarray):
    """parts: [ncores, N_CLASS, NOUT] fp32 device partials -> (loss, new_centers)."""
    tot = parts.astype(np.float64).sum(axis=0)
    S = tot[:, 0:D]
    SQ = tot[:, D : 2 * D]
    counts = tot[:, 2 * D]
    q = SQ.sum()
    c64 = centers.astype(np.float64)
    upd = ALPHA * (counts[:, None] * c64 - S) / (1.0 + counts[:, None])
    new_centers = (c64 - upd).astype(np.float32)
    loss = 0.5 * ((counts * (c64**2).sum(1)).sum() - 2.0 * (c64 * S).sum() + q)
    return np.float32(loss), new_centers


def kernel(preds, features, labels, centers):
    nc = _get_program()
    in_maps = []
    for k in range(NCORES):
        sl = slice(k * BS, (k + 1) * BS)
        in_maps.append(
            {
                "preds": np.ascontiguousarray(preds[sl], dtype=np.float32),
                "features": np.ascontiguousarray(features[sl], dtype=np.float32),
                "labels": np.ascontiguousarray(labels[sl], dtype=np.int32),
            }
        )
    res = run_bass_kernel_spmd(nc, in_maps, core_ids=list(range(NCORES)))
    parts = np.stack([res.results[k]["out"] for k in range(NCORES)])
    return combine_partials(parts, np.asarray(centers, dtype=np.float32))


# revision 33
# speedup vs baseline: 1.0342x; 1.0342x over previous
"""CenterCTCLoss Trainium2 kernel.

Strategy (data-parallel over batch, 8 cores, 512 rows each):
  The reference computation collapses to three segment statistics per core:
    counts[c] = sum of mask where labels==c
    S[c,d]    = sum of mask*features where labels==c
    SQ[c,d]   = sum of mask*features^2 where labels==c   (q = SQ.sum())
  where mask is the CTC char mask derived from argmax(preds).  Everything
  else (loss, centers update) is O(n_class*feat) math done on host from the
  8 per-core partial sums:
    upd[c]      = ALPHA*(counts[c]*centers[c] - S[c]) / (1+counts[c])
    new_centers = centers - upd
    loss        = 0.5*(sum_c counts[c]*|centers[c]|^2 - 2*<centers,S> + q)

  On device per chunk of 128 batch rows (batch on partitions), software-
  pipelined two stages deep so DMA/GPSIMD of chunk c+1 overlap the
  mask/scatter/matmul tail of chunk c:
    m[b,t]  = reduce_max over classes                     (DVE)
    dd      = bf16(preds - m) <= 0, == 0 only at argmax   (GPSIMD, exact)
    zmax[t] = max_c(dd_t + dd_{t+1})                      (DVE; == 0 iff the
              argmax repeats: a sum of two non-positives is 0 iff both are 0,
              which is exact in any float format)
    mask    = (dd[...,BLANK] != 0) * (zmax != 0)          (tiny)
    moh[p, (t%16)*85+lab] = mask[p,t]                     (GPSIMD local_scatter
              builds the masked one-hot; indices depend only on labels)
    psumA[85,64] += moh_t.T @ x_t   (bf16 PE, fp32 accum; one-hot and mask are
    psumB[85,65] += moh_t.T @ [x_t^2 | 1]    exact in bf16, features round)
"""

import sys

sys.path.insert(0, "/opt/trn_rl_repo")

from contextlib import ExitStack

import numpy as np

import concourse.bacc as bacc
import concourse.bass as bass
import concourse.mybir as mybir
import concourse.tile as tile
from concourse.bass_utils import run_bass_kernel_spmd

N_CLASS = 85
BLANK = 84
ALPHA = 0.05
B, T, D = 4096, 96, 64
NCORES = 8
BS = B // NCORES  # 512 batch rows per core
TH = T // 2  # half-chunk along t for DMA tiles

f32 = mybir.dt.float32
i32 = mybir.dt.int32
i16 = mybir.dt.int16
bf16 = mybir.dt.bfloat16

FDP = T * N_CLASS  # 8160 preds elems per row
NOUT = 2 * D + 1  # 129 output cols: S | SQ | counts


def build_program(bs: int = BS):
    nchunk = bs // 128
    nc = bacc.Bacc(
        "TRN2", target_bir_lowering=False, debug=False, num_devices=NCORES
    )
    preds = nc.dram_tensor("preds", [bs, T, N_CLASS], f32, kind="ExternalInput").ap()
    feats = nc.dram_tensor("features", [bs, T, D], f32, kind="ExternalInput").ap()
    labels = nc.dram_tensor("labels", [bs, T], i32, kind="ExternalInput").ap()
    out = nc.dram_tensor("out", [N_CLASS, NOUT], f32, kind="ExternalOutput").ap()

    CP = N_CLASS + 1  # 86: class dim padded so bf16 shifted views stay 4B-aligned

    with tile.TileContext(nc, pool_alloc_mode="queue") as tc, ExitStack() as ctx:
        consts = ctx.enter_context(tc.tile_pool(name="consts", bufs=1))
        ppool = ctx.enter_context(tc.tile_pool(name="ppool", bufs=8))
        xpool = ctx.enter_context(tc.tile_pool(name="xpool", bufs=2))
        work = ctx.enter_context(tc.tile_pool(name="work", bufs=1))
        work2 = ctx.enter_context(tc.tile_pool(name="work2", bufs=2))
        mmin = ctx.enter_context(tc.tile_pool(name="mmin", bufs=2))
        small = ctx.enter_context(tc.tile_pool(name="small", bufs=4))
        psum_pool = ctx.enter_context(
            tc.tile_pool(name="psum", bufs=1, space="PSUM")
        )

        # per-t scatter constant: tb85[p,t] = (t%16)*85
        TB = 16  # t-block size for local_scatter (16*85=1360 elems < 2047)
        NB = T // TB
        tb85i = consts.tile([128, T], i16)
        nc.gpsimd.iota(
            tb85i[:], pattern=[[0, NB], [N_CLASS, TB]], base=0, channel_multiplier=0
        )
        tb85f = consts.tile([128, T], f32)
        nc.scalar.copy(tb85f[:], tb85i[:])

        psumA = psum_pool.tile([N_CLASS, D], f32)  # S
        psumB = psum_pool.tile([N_CLASS, D + 1], f32)  # SQ | counts

        def repdet_half(st, half):
            # repeated argmax for one t-half of the previous chunk:
            # zmax[t] = max_c(dd_t + dd_{t+1}) == 0 iff argmax repeats (a sum
            # of two non-positives is 0 iff both are 0 -- exact in bf16).
            # The shifted add runs in place over dd: in1 reads stay 86
            # elements ahead of the writes, so old values are consumed first.
            ddf = st["dd"][:].rearrange("p t c -> p (t c)")
            nhalf = (T - 1) // 2  # 47; halves are 47 and 48 wide
            w0, w1 = ((0, nhalf), (nhalf, T - 1))[half]
            nc.vector.tensor_add(
                ddf[:, w0 * CP : w1 * CP],
                ddf[:, w0 * CP : w1 * CP],
                ddf[:, (w0 + 1) * CP : (w1 + 1) * CP],
            )
            nc.vector.tensor_reduce(
                st["zmax"][:, w0:w1],
                ddf[:, w0 * CP : w1 * CP].rearrange("p (t c) -> p t c", c=CP),
                axis=mybir.AxisListType.X,
                op=mybir.AluOpType.max,
            )

        def stage1a(c, prev):
            brange = slice(c * 128, (c + 1) * 128)
            # preds path: load; dd[p,t,c] = preds - max <= 0, 0 only at argmax
            dd = work2.tile([128, T, CP], bf16, tag="dd")
            m = small.tile([128, T], f32, tag="m")
            nc.gpsimd.memset(dd[:, :, N_CLASS], -1e30)
            TQ = T // 4
            for h in range(4):
                ts_ = slice(h * TQ, (h + 1) * TQ)
                ph = ppool.tile([128, TQ, N_CLASS], f32, tag="ph")
                nc.sync.dma_start(ph[:], preds[brange, ts_, :])

                nc.vector.tensor_reduce(
                    m[:, ts_], ph[:], axis=mybir.AxisListType.X, op=mybir.AluOpType.max
                )
                mb = m[:, ts_].unsqueeze(2).broadcast_to([128, TQ, N_CLASS])
                nc.gpsimd.tensor_tensor(
                    dd[:, ts_, 0:N_CLASS], ph[:], mb, op=mybir.AluOpType.subtract
                )
                # interleave the previous chunk's repeat-detection halves so
                # the DVE has ready work while this chunk's DMAs land
                if prev is not None and h in (1, 3):
                    repdet_half(prev, h // 2)
            # blank-argmax indicator only needs dd
            inv84 = small.tile([128, T], f32, tag="inv84")
            nc.vector.tensor_single_scalar(
                inv84[:], dd[:, :, BLANK], 0.0, op=mybir.AluOpType.not_equal
            )
            zmax = small.tile([128, T], f32, tag="zmax")
            nc.vector.memset(zmax[:, T - 1 : T], -1.0)
            return dict(dd=dd, inv84=inv84, zmax=zmax)

        def stage1b(c, st):
            brange = slice(c * 128, (c + 1) * 128)
            # features/labels path, only needed by the stage2 matmuls
            xb = mmin.tile([128, T, D], bf16, tag="xb")
            xsq = mmin.tile([128, T, D + 1], bf16, tag="xsq")
            labI = small.tile([128, T], i32, tag="labI")
            nc.sync.dma_start(labI[:], labels[brange, :])
            for h in range(2):
                ts_ = slice(h * TH, (h + 1) * TH)
                xh = xpool.tile([128, TH, D], f32, tag="xh")
                nc.sync.dma_start(xh[:], feats[brange, ts_, :])
                # features: bf16 copy and squares (ScalarE)
                nc.scalar.copy(xb[:, ts_, :], xh[:])
                nc.scalar.square(xsq[:, ts_, 0:D], xh[:])
            nc.vector.memset(xsq[:, :, D], 1.0)
            labf = small.tile([128, T], f32, tag="labf")
            nc.scalar.copy(labf[:], labI[:])
            idx16 = small.tile([128, T], i16, tag="idx16")
            nc.vector.tensor_add(idx16[:], tb85f[:], labf[:])
            st.update(xb=xb, xsq=xsq, idx16=idx16)
            return st

        def stage2(c, st):
            xb, xsq = st["xb"], st["xsq"]
            inv84, idx16, zmax = st["inv84"], st["idx16"], st["zmax"]

            # mask = (dd[..,BLANK] != 0) * (zmax != 0), scattered as DATA:
            # moh[p, (t%16)*85+lab] = mask[p,t] (indices never masked, always
            # distinct within a 16-t block)
            maskb = small.tile([128, T], bf16, tag="maskb")
            nc.vector.scalar_tensor_tensor(
                maskb[:], zmax[:], 0.0, inv84[:],
                op0=mybir.AluOpType.not_equal, op1=mybir.AluOpType.mult,
            )

            # masked one-hot (bf16) via gpsimd local scatter, one tile per
            # 16-t block so each block's PE matmuls start as soon as its own
            # scatter lands instead of waiting for all six
            for blk in range(NB):
                moh = work.tile([128, TB * N_CLASS], bf16, tag=f"moh{blk}")
                nc.gpsimd.local_scatter(
                    moh[:],
                    maskb[:, blk * TB : (blk + 1) * TB],
                    idx16[:, blk * TB : (blk + 1) * TB],
                    channels=128,
                    num_elems=TB * N_CLASS,
                    num_idxs=TB,
                )
                m3 = moh[:].rearrange("p (t c) -> p t c", c=N_CLASS)
                for tt in range(TB):
                    t = blk * TB + tt
                    first = c == 0 and t == 0
                    last = c == nchunk - 1 and t == T - 1
                    nc.tensor.matmul(
                        psumA[:], m3[:, tt, :], xb[:, t, :], start=first, stop=last
                    )
                    nc.tensor.matmul(
                        psumB[:], m3[:, tt, :], xsq[:, t, :], start=first, stop=last
                    )

        # software pipeline: the preds path of chunk c+1 (DMA, max-reduce,
        # gpsimd subtract) is emitted before stage2 of chunk c so it queues
        # ahead of the heavy DVE tail work; the features path follows.
        pending = None
        for c in range(nchunk + 1):
            if c < nchunk:
                st = stage1a(c, pending)
            else:
                for half in (0, 1):
                    repdet_half(pending, half)
            if pending is not None:
                stage2(c - 1, pending)
            if c < nchunk:
                stage1b(c, st)
            pending = st if c < nchunk else None

        outT = consts.tile([N_CLASS, NOUT], f32)
        nc.scalar.copy(outT[:, 0:D], psumA[:])
        nc.scalar.copy(outT[:, D : 2 * D + 1], psumB[:])
        nc.sync.dma_start(out[:], outT[:])

    nc.compile()
    return nc


_prog_cache: dict[int, object] = {}


def _get_program(bs: int = BS):
    if bs not in _prog_cache:
        _prog_cache[bs] = build_program(bs)
    return _prog_cache[bs]


def combine_partials(parts: np.ndarray, centers: np.ndarray):
    """parts: [ncores, N_CLASS, NOUT] fp32 device partials -> (loss, new_centers)."""
    tot = parts.astype(np.float64).sum(axis=0)
    S = tot[:, 0:D]
    SQ = tot[:, D : 2 * D]
    counts = tot[:, 2 * D]
    q = SQ.sum()
    c64 = centers.astype(np.float64)
    upd = ALPHA * (counts[:, None] * c64 - S) / (1.0 + counts[:, None])
    new_centers = (c64 - upd).astype(np.float32)
    loss = 0.5 * ((counts * (c64**2).sum(1)).sum() - 2.0 * (c64 * S).sum() + q)
    return np.float32(loss), new_centers


def kernel(preds, features, labels, centers):
    nc = _get_program()
    in_maps = []
    for k in range(NCORES):
        sl = slice(k * BS, (k + 1) * BS)
        in_maps.append(
            {
                "preds": np.ascontiguousarray(preds[sl], dtype=np.float32),
                "features": np.ascontiguousarray(features[sl], dtype=np.float32),
                "labels": np.ascontiguousarray(labels[sl], dtype=np.int32),
            }
        )
    res = run_bass_kernel_spmd(nc, in_maps, core_ids=list(range(NCORES)))
    parts = np.stack([res.results[k]["out"] for k in range(NCORES)])
    return combine_partials(parts, np.asarray(centers, dtype=np.float32))
